# revision 36
# baseline (speedup 1.0000x reference)
"""Trainium2 Bass kernel for nn_Encoder (dense_cnn).

Full-input contract: kernel(**inputs) takes the unsharded numpy inputs and
returns (out [B,1,2S], attn [B,1,D-1], recon [B,D,L]) matching the fp32
reference.

Sharding across 8 NeuronCores:
  - W_attn [S,S] is sharded column-wise: core c gets columns [1008c, 1008c+1008),
    with rows permuted to s' = 64p + h so the lhsT (mainT) tiles are buildable
    from the pooled conv output with two strided DVE copies (no transpose DMA).
  - Convs are computed where needed: each core computes main (d=0, all h)
    fully, the aux slice for its h-range [8c, 8c+8) x all d, and the recon
    path for its 4 batches [4c, 4c+4).
  - Partial attention scores are folded to [15,32] on PE and AllReduced;
    softmax is replicated; each core emits its weighted slice [B, 1008].
All matmuls run as float32r (full PE rate at N>=256, tf32-class mantissa).
"""

import numpy as np
import ml_dtypes
from contextlib import ExitStack

import concourse.bass as bass
import concourse.bacc as bacc
import concourse.tile as tile
import concourse.mybir as mybir
from concourse.bass_utils import run_bass_kernel_spmd

F32 = mybir.dt.float32
F32R = mybir.dt.float32r
AX = mybir.AxisListType
ALU = mybir.AluOpType
ACTF = mybir.ActivationFunctionType

B, L, D, H, KK = 32, 256, 16, 64, 5
T = L - KK + 1          # 252
P = T // 2              # 126
S = P * H               # 8064
NC = 8                  # cores
HSH = H // NC           # 8 h-values per core
SSH = S // NC           # 1008 columns of W / weighted per core
BSH = B // NC           # 4 batches per core for recon
DK = D * KK             # 80  (k,d) partitions for conv im2col
DH = D * H              # 1024 (d,h) partitions for recon conv
NKC = S // 128          # 63 contraction chunks for the big matmul

_CACHED = {}


def build_nc(w_bufs=2):
    nc = bacc.Bacc(None, num_devices=NC)

    # ---- external inputs (per core) ----
    xrep_i = nc.dram_tensor("xrep_i", [DK, B, T], F32R, kind="ExternalInput")
    xrep0_i = nc.dram_tensor("xrep0_i", [KK, B, T], F32R, kind="ExternalInput")
    xrrep_i = nc.dram_tensor("xrrep_i", [DK, BSH, T], F32R, kind="ExternalInput")
    w_aux = nc.dram_tensor("w_aux", [DK, 128], F32R, kind="ExternalInput")
    w_main = nc.dram_tensor("w_main", [KK, H], F32R, kind="ExternalInput")
    w_rec = nc.dram_tensor("w_rec", [DK, DH], F32R, kind="ExternalInput")
    w_dec = nc.dram_tensor("w_dec", [128, 8, DK], F32R, kind="ExternalInput")
    cb_aux = nc.dram_tensor("cb_aux", [128, 1], F32, kind="ExternalInput")
    cb_main = nc.dram_tensor("cb_main", [H, 1], F32, kind="ExternalInput")
    cb_rec = nc.dram_tensor("cb_rec", [128, 8], F32, kind="ExternalInput")
    dec_b = nc.dram_tensor("dec_b", [D, 1], F32, kind="ExternalInput")
    sel8 = nc.dram_tensor("sel8", [128, HSH], F32R, kind="ExternalInput")
    sel_d = nc.dram_tensor("sel_d", [D - 1, 128], F32R, kind="ExternalInput")
    sel_f = nc.dram_tensor("sel_f", [128, D - 1], F32R, kind="ExternalInput")
    sel_k = nc.dram_tensor("sel_k", [DK, KK, D], F32R, kind="ExternalInput")
    sel_r = nc.dram_tensor("sel_r", [HSH, 128], F32R, kind="ExternalInput")
    BF16 = mybir.dt.bfloat16
    Wsh = nc.dram_tensor("Wsh", [128, NKC, SSH], BF16, kind="ExternalInput")

    # ---- external outputs (per core) ----
    main_out = nc.dram_tensor("main_out", [H, B, P], F32, kind="ExternalOutput")
    weighted_out = nc.dram_tensor("weighted_out", [HSH, B, P], F32, kind="ExternalOutput")
    attn_out = nc.dram_tensor("attn_out", [B, D - 1], F32, kind="ExternalOutput")
    recon_out = nc.dram_tensor("recon_out", [D, BSH, L], F32, kind="ExternalOutput")

    # ---- internal DRAM scratch ----
    m_d = nc.dram_tensor("m_d", [B, SSH], F32R)
    attnT_d = nc.dram_tensor("attnT_d", [D - 1, B], F32R)
    cc_in = nc.dram_tensor("cc_in", [D - 1, B], F32)
    cc_out = nc.dram_tensor("cc_out", [D - 1, B], F32, addr_space="Shared")

    with tile.TileContext(nc) as tc, ExitStack() as ctx:
        const = ctx.enter_context(tc.tile_pool(name="const", bufs=1))
        big = ctx.enter_context(tc.tile_pool(name="big", bufs=1))
        wpool = ctx.enter_context(tc.tile_pool(name="wpool", bufs=w_bufs))
        ypool = ctx.enter_context(tc.tile_pool(name="ypool", bufs=2))
        yrpool = ctx.enter_context(tc.tile_pool(name="yrpool", bufs=2))
        wtpool = ctx.enter_context(tc.tile_pool(name="wtpool", bufs=2))
        ps_conv = ctx.enter_context(tc.tile_pool(name="ps_conv", bufs=2, space="PSUM"))
        ps_mm = ctx.enter_context(tc.tile_pool(name="ps_mm", bufs=1, space="PSUM"))
        ps_z = ctx.enter_context(tc.tile_pool(name="ps_z", bufs=1, space="PSUM"))
        ps_rec = ctx.enter_context(tc.tile_pool(name="ps_rec", bufs=1, space="PSUM"))
        ps_wt = ctx.enter_context(tc.tile_pool(name="ps_wt", bufs=1, space="PSUM"))

        # ---------- conv-main inputs first: they gate the whole chain ----------
        xrep0 = big.tile([KK, B, T], F32R)
        nc.sync.dma_start(xrep0[:], xrep0_i[:])
        wmain_sb = const.tile([KK, H], F32R)
        nc.sync.dma_start(wmain_sb[:], w_main[:])
        cbm_sb = const.tile([H, 1], F32)
        nc.sync.dma_start(cbm_sb[:], cb_main[:])
        waux_sb = const.tile([DK, 128], F32R)
        nc.sync.dma_start(waux_sb[:], w_aux[:])
        wrec_sb = const.tile([DK, DH], F32R)
        nc.sync.dma_start(wrec_sb[:], w_rec[:])
        wdec_sb = const.tile([128, 8, DK], F32R)
        nc.sync.dma_start(wdec_sb[:], w_dec[:])
        cba_sb = const.tile([128, 1], F32)
        nc.sync.dma_start(cba_sb[:], cb_aux[:])
        cbr_sb = const.tile([128, 8], F32)
        nc.sync.dma_start(cbr_sb[:], cb_rec[:])
        decb_sb = const.tile([D, 1], F32)
        nc.sync.dma_start(decb_sb[:], dec_b[:])
        sel8_sb = const.tile([128, HSH], F32R)
        nc.sync.dma_start(sel8_sb[:], sel8[:])
        seld_sb = const.tile([D - 1, 128], F32R)
        nc.sync.dma_start(seld_sb[:], sel_d[:])
        self_sb = const.tile([128, D - 1], F32R)
        nc.sync.dma_start(self_sb[:], sel_f[:])
        selk_sb = const.tile([DK, KK, D], F32R)
        nc.sync.dma_start(selk_sb[:], sel_k[:])
        selr_sb = const.tile([HSH, 128], F32R)
        nc.sync.dma_start(selr_sb[:], sel_r[:])

        # ---------- im2col inputs (host-prepared, contiguous loads) ----------
        # on scalar so sync can start streaming W immediately
        xrep = big.tile([DK, B, T], F32R)
        nc.scalar.dma_start(xrep[:], xrep_i[:])
        xrrep = big.tile([DK, BSH, T], F32R)
        nc.scalar.dma_start(xrrep[:], xrrep_i[:])

        # ---------- conv main (d=0, all h) ----------
        # pools write pm (fp32, for main_out) and mainT (bf16 lhsT) directly
        pm = big.tile([H, B, P], F32)       # pooled main, (b,p) free order
        mainT_sb = big.tile([128, NKC, B], BF16)
        NB = 2                              # batches per conv chunk
        for c in range(B // NB):
            ps = ps_conv.tile([128, NB * T], F32, tag="ps")
            nc.tensor.matmul(ps[:H, :], wmain_sb[:],
                             xrep0[:, NB * c:NB * (c + 1), :],
                             start=True, stop=True)
            ym = ypool.tile([H, NB, T], F32, tag="ym")
            nc.scalar.activation(ym[:], ps[:H, :].rearrange("h (b t) -> h b t", b=NB),
                                 ACTF.Relu, bias=cbm_sb[:])
            nc.vector.tensor_tensor(pm[:, NB * c:NB * (c + 1), :],
                                    ym[:, :, 0:T - 1:2], ym[:, :, 1:T:2], op=ALU.add)
            # mainT[s'=64p+h]: even p -> partitions 0:64, odd p -> 64:128
            nc.vector.tensor_tensor(
                mainT_sb[0:64, :, NB * c:NB * (c + 1)].transpose([0, 2, 1]),
                ym[:, :, 0:T - 1:4], ym[:, :, 1:T:4], op=ALU.add)
            nc.vector.tensor_tensor(
                mainT_sb[64:128, :, NB * c:NB * (c + 1)].transpose([0, 2, 1]),
                ym[:, :, 2:T:4], ym[:, :, 3:T:4], op=ALU.add)

        # ---------- conv aux (all d, this core's 8 h) ----------
        pa = big.tile([128, B, P], F32R)    # pooled aux [(d,h8), b, p]
        for c in range(B // NB):
            ps = ps_conv.tile([128, NB * T], F32, tag="ps")
            nc.tensor.matmul(ps[:], waux_sb[:],
                             xrep[:, NB * c:NB * (c + 1), :],
                             start=True, stop=True)
            ya = ypool.tile([128, NB, T], F32, tag="ya")
            nc.scalar.activation(ya[:], ps[:].rearrange("h (b t) -> h b t", b=NB),
                                 ACTF.Relu, bias=cba_sb[:])
            nc.vector.tensor_tensor(pa[:, NB * c:NB * (c + 1), :],
                                    ya[:, :, 0:T - 1:2], ya[:, :, 1:T:2], op=ALU.add)

        # ---------- big matmul: m = main @ Wsh ----------
        m_ps0 = ps_mm.tile([B, 504], F32, tag="m0")
        m_ps1 = ps_mm.tile([B, 504], F32, tag="m1")
        dma_engines = [nc.sync, nc.gpsimd]
        WMAC = 8                            # k-chunks per W macro-tile DMA
        kc = 0
        ti = 0
        while kc < NKC:
            nmac = min(WMAC, NKC - kc)
            wt = wpool.tile([128, WMAC, SSH], BF16, tag="w")
            dma_engines[ti % 2].dma_start(
                wt[:, 0:nmac, :], Wsh[:, kc:kc + nmac, :])
            for j in range(nmac):
                nc.tensor.matmul(m_ps0[:], mainT_sb[:, kc + j, :],
                                 wt[:, j, 0:504],
                                 start=(kc + j == 0), stop=(kc + j == NKC - 1))
                nc.tensor.matmul(m_ps1[:], mainT_sb[:, kc + j, :],
                                 wt[:, j, 504:SSH],
                                 start=(kc + j == 0), stop=(kc + j == NKC - 1))
            kc += nmac
            ti += 1
        nc.sync.dma_start(main_out[:], pm[:])
        m_sb = big.tile([B, SSH], F32R)
        nc.scalar.copy(m_sb[:, 0:504], m_ps0[:])
        nc.scalar.copy(m_sb[:, 504:SSH], m_ps1[:])

        # ---------- scores partials + AllReduce ----------
        # m8[h8, b, p] = m[b, h8*P + p]; PE-broadcast to all 16 d-slots
        nc.sync.dma_start(m_d[:], m_sb[:])
        m8 = big.tile([HSH, B, P], F32R)
        nc.sync.dma_start(m8[:], m_d[:].rearrange("b (h p) -> h b p", h=HSH))
        m_rep = big.tile([128, B, P], F32)
        m8f = m8[:].rearrange("h b p -> h (b p)")
        for c8 in range(8):
            mr_ps = ps_wt.tile([128, 504], F32, tag="wt")
            nc.tensor.matmul(mr_ps[:], selr_sb[:], m8f[:, 504 * c8:504 * (c8 + 1)],
                             start=True, stop=True)
            nc.scalar.copy(
                m_rep[:].rearrange("q b p -> q (b p)")[:, 504 * c8:504 * (c8 + 1)],
                mr_ps[:])
        nc.vector.tensor_tensor(m_rep[:], m_rep[:], pa[:], op=ALU.mult)
        sc_hb = const.tile([128, B], F32R)
        with nc.allow_low_precision(reason="f32r rounding of fp32-accumulated scores"):
            nc.vector.tensor_reduce(sc_hb[:], m_rep[:], axis=AX.X, op=ALU.add)
        # fold h8 on PE: scT[d', b] = sum_{(d,h8)} sel_f * sc_hb
        sc_ps = ps_wt.tile([D - 1, B], F32, tag="wt")
        nc.tensor.matmul(sc_ps[:], self_sb[:], sc_hb[:], start=True, stop=True)
        scT_sb = const.tile([D - 1, B], F32)
        nc.scalar.copy(scT_sb[:], sc_ps[:])
        nc.sync.dma_start(cc_in[:], scT_sb[:])
        nc.gpsimd.collective_compute(
            "AllReduce", ALU.add, replica_groups=[list(range(NC))],
            ins=[cc_in[:]], outs=[cc_out[:]])

        # ---------- recon path (fills the collective window) ----------
        z_sb = big.tile([DK, BSH, L + 4], F32R)   # zero-padded: z at t+4
        # f32r memset lacks an ISA encoding; zero the margins via ACT scale=0
        nc.scalar.activation(z_sb[:, :, 0:4], selk_sb[:, 0:4, 0:4],
                             ACTF.Copy, scale=0.0)
        nc.scalar.activation(z_sb[:, :, 256:260], selk_sb[:, 0:4, 0:4],
                             ACTF.Copy, scale=0.0)
        xrf = xrrep[:].rearrange("a b t -> a (b t)")
        for h2 in range(2):
            z_ps = ps_z.tile([DK, 504], F32, tag="z")
            for j in range(8):
                ps = ps_conv.tile([128, 504], F32, tag="ps")
                nc.tensor.matmul(ps[:], wrec_sb[:, 128 * j:128 * (j + 1)],
                                 xrf[:, 504 * h2:504 * (h2 + 1)],
                                 start=True, stop=True)
                yr = yrpool.tile([128, 504], F32R, tag="yr")
                nc.scalar.activation(yr[:], ps[:], ACTF.Relu,
                                     bias=cbr_sb[:, j:j + 1])
                nc.tensor.matmul(z_ps[:], wdec_sb[:, j, :], yr[:],
                                 start=(j == 0), stop=(j == 7))
            nc.scalar.copy(z_sb[:, 2 * h2:2 * h2 + 2, 4:4 + T],
                           z_ps[:].rearrange("a (b t) -> a b t", b=2))
        # k-fold via accumulating selector matmuls over shifted input windows
        rec_ps = ps_rec.tile([D, BSH, L], F32)
        for b4 in range(BSH):
            for k in range(KK):
                nc.tensor.matmul(rec_ps[:, b4, :], selk_sb[:, k, :],
                                 z_sb[:, b4, 4 - k:4 - k + L],
                                 start=(k == 0), stop=(k == KK - 1),
                                 skip_group_check=True)
        rec_sb = big.tile([D, BSH, L], F32)
        nc.vector.tensor_scalar_add(rec_sb[:], rec_ps[:], decb_sb[:])
        nc.sync.dma_start(recon_out[:], rec_sb[:])

        # ---------- softmax ----------
        scores = const.tile([B, D - 1], F32)
        nc.sync.dma_start(scores[:], cc_out[:].transpose([1, 0]))
        ex = const.tile([B, D - 1], F32)
        esum = const.tile([B, 1], F32)
        nc.scalar.activation(ex[:], scores[:], ACTF.Exp, accum_out=esum[:])
        rs = const.tile([B, 1], F32)
        nc.vector.reciprocal(rs[:], esum[:])
        attn_sb = const.tile([B, D - 1], F32)
        nc.vector.tensor_scalar_mul(attn_sb[:], ex[:], rs[:])
        nc.sync.dma_start(attn_out[:], attn_sb[:])
        nc.sync.dma_start(attnT_d[:].transpose([1, 0]), attn_sb[:].bitcast(F32R))
        attnT_sb = const.tile([D - 1, B], F32R)
        nc.sync.dma_start(attnT_sb[:], attnT_d[:])

        # ---------- weighted = sum_d attn[b,d] * aux ----------
        arep_ps = ps_conv.tile([128, B], F32, tag="ps")
        nc.tensor.matmul(arep_ps[:], seld_sb[:], attnT_sb[:], start=True, stop=True)
        arep_sb = const.tile([128, B], F32)
        nc.scalar.copy(arep_sb[:], arep_ps[:])
        # wsc = aux * attn (broadcast along p), chunked and pipelined
        NBW = 4                             # batches per selector-matmul chunk
        for c in range(B // NBW):
            nc.vector.tensor_tensor(
                pa[:, NBW * c:NBW * (c + 1), :], pa[:, NBW * c:NBW * (c + 1), :],
                arep_sb[:].unsqueeze(-1).broadcast_to([128, B, P])
                [:, NBW * c:NBW * (c + 1), :], op=ALU.mult)
            wt_ps = ps_wt.tile([HSH, NBW * P], F32, tag="wt")
            nc.tensor.matmul(wt_ps[:], sel8_sb[:],
                             pa[:, NBW * c:NBW * (c + 1), :],
                             start=True, stop=True)
            wchunk = wtpool.tile([HSH, NBW, P], F32, tag="wc")
            nc.scalar.copy(wchunk[:], wt_ps[:].rearrange("h (b p) -> h b p", b=NBW))
            nc.sync.dma_start(weighted_out[:, NBW * c:NBW * (c + 1), :], wchunk[:])

    nc.compile()
    return nc


def _prep_inputs(x, conv_w, conv_b, deconv_w, deconv_b, W_attn):
    """Build the per-core input maps (all numpy float32)."""
    x = np.ascontiguousarray(np.asarray(x, np.float32))
    conv_w = np.asarray(conv_w, np.float32)
    conv_b = np.asarray(conv_b, np.float32)
    deconv_w = np.asarray(deconv_w, np.float32)
    deconv_b = np.asarray(deconv_b, np.float32)
    W_attn = np.asarray(W_attn, np.float32)

    xT = np.ascontiguousarray(x.transpose(2, 0, 1))          # [D, B, L]
    # im2col stacks: xrep[(k,d), b, t] = x[b, t+k, d]
    xrep_i = np.ascontiguousarray(np.concatenate(
        [xT[:, :, k:k + T] for k in range(KK)], axis=0).reshape(DK, B, T))
    xrep0_i = np.ascontiguousarray(
        np.stack([xT[0, :, k:k + T] for k in range(KK)], axis=0))
    # rows permuted: s' = 64p + h  <->  s = 126h + p
    W_perm = np.ascontiguousarray(
        W_attn.reshape(H, P, S).transpose(1, 0, 2).reshape(S, S))

    # conv stationaries, (k,d) partition order; aux/main scaled by 0.5
    kd = np.zeros((DK, D, H), np.float32)                    # [(k,d), d2, h]
    for k in range(KK):
        for d in range(D):
            kd[16 * k + d, d, :] = conv_w[d, :, k]
    w_main = np.ascontiguousarray(0.5 * conv_w[0].T)          # [5, 64]
    w_rec = np.ascontiguousarray(kd.reshape(DK, DH))          # [80, 1024]

    wdec = np.zeros((DH, DK), np.float32)
    for d in range(D):
        for k in range(KK):
            wdec[d * H:(d + 1) * H, 16 * k + d] = deconv_w[d, :, k]
    w_dec = np.ascontiguousarray(wdec.reshape(8, 128, DK).transpose(1, 0, 2))

    cb_main = np.ascontiguousarray(0.5 * conv_b[0, :, None])  # [64,1]
    cb_rec = np.ascontiguousarray(conv_b.reshape(DH).reshape(8, 128).T)
    dec_b = np.ascontiguousarray(deconv_b[:, None])           # [16,1]

    # sel8[(d,h8), h8'] = [h8==h8'] for d>=1 else 0 -> [128, 8]
    sel8 = np.ascontiguousarray(np.concatenate(
        [np.zeros((HSH, HSH), np.float32),
         np.tile(np.eye(HSH, dtype=np.float32), (D - 1, 1))], axis=0))
    # sel_d[d', (d,h8)] = [d==d'+1] -> [15, 128]
    sel_d = np.zeros((D - 1, 128), np.float32)
    for dp in range(D - 1):
        sel_d[dp, (dp + 1) * HSH:(dp + 2) * HSH] = 1.0
    # sel_f[(d,h8), d'] = [d==d'+1] -> [128, 15]
    sel_f = np.ascontiguousarray(sel_d.T)
    # sel_r[h8', (d,h8)] = [h8==h8'] -> [8, 128] (PE broadcast of m8)
    sel_r = np.ascontiguousarray(np.tile(np.eye(HSH, dtype=np.float32), (1, D)))
    # sel_k[(k',d'), k, d] = [k'==k][d'==d] -> [80, 5, 16]
    sel_k = np.zeros((DK, KK, D), np.float32)
    for k in range(KK):
        for d in range(D):
            sel_k[16 * k + d, k, d] = 1.0

    in_maps = []
    for c in range(NC):
        wa = np.zeros((DK, 128), np.float32)
        for k in range(KK):
            for d in range(D):
                wa[16 * k + d, d * HSH:(d + 1) * HSH] = \
                    0.5 * conv_w[d, HSH * c:HSH * (c + 1), k]
        cb_aux = np.ascontiguousarray(
            0.5 * conv_b[:, HSH * c:HSH * (c + 1)].reshape(128)[:, None])
        xslice = xT[:, BSH * c:BSH * (c + 1), :]
        xrrep_i = np.ascontiguousarray(np.concatenate(
            [xslice[:, :, k:k + T] for k in range(KK)], axis=0).reshape(DK, BSH, T))
        in_maps.append({
            "xrep_i": xrep_i,
            "xrep0_i": xrep0_i,
            "xrrep_i": xrrep_i,
            "w_aux": wa,
            "w_main": w_main,
            "w_rec": w_rec,
            "w_dec": w_dec,
            "cb_aux": cb_aux,
            "cb_main": cb_main,
            "cb_rec": cb_rec,
            "dec_b": dec_b,
            "sel8": sel8,
            "sel_d": sel_d,
            "sel_f": sel_f,
            "sel_k": sel_k,
            "sel_r": sel_r,
            "Wsh": np.ascontiguousarray(
                W_perm[:, SSH * c:SSH * (c + 1)].astype(ml_dtypes.bfloat16)
                .reshape(NKC, 128, SSH).transpose(1, 0, 2)),
        })
    return in_maps


def _assemble(results):
    main = np.ascontiguousarray(
        results[0]["main_out"].transpose(1, 0, 2)).reshape(B, S)
    weighted = np.empty((B, H, P), np.float32)
    for c, r in enumerate(results):
        weighted[:, HSH * c:HSH * (c + 1), :] = r["weighted_out"].transpose(1, 0, 2)
    weighted = weighted.reshape(B, S)
    attn = results[0]["attn_out"][:, None, :]                  # [B,1,15]
    recon = np.concatenate(
        [r["recon_out"].transpose(1, 0, 2) for r in results], axis=0)
    out = np.concatenate([main[:, None, :], weighted[:, None, :]], axis=2)
    return out, attn, recon


def kernel(x, conv_w, conv_b, deconv_w, deconv_b, W_attn,
           _trace=False, _trace_kwargs=None):
    if "nc" not in _CACHED:
        _CACHED["nc"] = build_nc()
    nc = _CACHED["nc"]
    in_maps = _prep_inputs(x, conv_w, conv_b, deconv_w, deconv_b, W_attn)
    res = run_bass_kernel_spmd(nc, in_maps, list(range(NC)), trace=_trace,
                               **(_trace_kwargs or {}))
    outs = _assemble(res.results)
    if _trace:
        return outs, res
    return outs


if __name__ == "__main__":
    d = np.load("/root/problem/ref_cache.npz")
    out, attn, recon = kernel(d["x"], d["conv_w"], d["conv_b"],
                              d["deconv_w"], d["deconv_b"], d["W_attn"])
    for name, a, b in [("out", out, d["out"]), ("attn", attn, d["attn"]),
                       ("recon", recon, d["recon"])]:
        err = np.abs(a - b).max() / (np.abs(b).max() + 1e-30)
        print(f"{name}: rel_err {err:.3e}")


# revision 39
# speedup vs baseline: 1.0748x; 1.0748x over previous
"""Trainium2 Bass kernel for nn_Encoder (dense_cnn).

Full-input contract: kernel(**inputs) takes the unsharded numpy inputs and
returns (out [B,1,2S], attn [B,1,D-1], recon [B,D,L]) matching the fp32
reference.

Sharding across 8 NeuronCores:
  - W_attn [S,S] is sharded column-wise: core c gets columns [1008c, 1008c+1008),
    with rows permuted to s' = 64p + h so the lhsT (mainT) tiles are buildable
    from the pooled conv output with two strided DVE copies (no transpose DMA).
  - Convs are computed where needed: each core computes main (d=0, all h)
    fully, the aux slice for its h-range [8c, 8c+8) x all d, and the recon
    path for its 4 batches [4c, 4c+4).
  - Partial attention scores are folded to [15,32] on PE and AllReduced;
    softmax is replicated; each core emits its weighted slice [B, 1008].
All matmuls run as float32r (full PE rate at N>=256, tf32-class mantissa).
"""

import numpy as np
import ml_dtypes
from contextlib import ExitStack

import concourse.bass as bass
import concourse.bacc as bacc
import concourse.tile as tile
import concourse.mybir as mybir
from concourse.bass_utils import run_bass_kernel_spmd

F32 = mybir.dt.float32
F32R = mybir.dt.float32r
AX = mybir.AxisListType
ALU = mybir.AluOpType
ACTF = mybir.ActivationFunctionType

B, L, D, H, KK = 32, 256, 16, 64, 5
T = L - KK + 1          # 252
P = T // 2              # 126
S = P * H               # 8064
NC = 8                  # cores
HSH = H // NC           # 8 h-values per core
SSH = S // NC           # 1008 columns of W / weighted per core
BSH = B // NC           # 4 batches per core for recon
DK = D * KK             # 80  (k,d) partitions for conv im2col
DH = D * H              # 1024 (d,h) partitions for recon conv
NKC = S // 128          # 63 contraction chunks for the big matmul

_CACHED = {}


def build_nc(w_bufs=2):
    nc = bacc.Bacc(None, num_devices=NC)

    # ---- external inputs (per core) ----
    xrep_i = nc.dram_tensor("xrep_i", [DK, B, T], F32R, kind="ExternalInput")
    xrep0_i = nc.dram_tensor("xrep0_i", [KK, B, T], F32R, kind="ExternalInput")
    xrrep_i = nc.dram_tensor("xrrep_i", [DK, BSH, T], F32R, kind="ExternalInput")
    w_aux = nc.dram_tensor("w_aux", [DK, 128], F32R, kind="ExternalInput")
    w_main = nc.dram_tensor("w_main", [KK, H], F32R, kind="ExternalInput")
    w_rec = nc.dram_tensor("w_rec", [DK, DH], F32R, kind="ExternalInput")
    w_dec = nc.dram_tensor("w_dec", [128, 8, DK], F32R, kind="ExternalInput")
    cb_aux = nc.dram_tensor("cb_aux", [128, 1], F32, kind="ExternalInput")
    cb_main = nc.dram_tensor("cb_main", [H, 1], F32, kind="ExternalInput")
    cb_rec = nc.dram_tensor("cb_rec", [128, 8], F32, kind="ExternalInput")
    dec_b = nc.dram_tensor("dec_b", [D, 1], F32, kind="ExternalInput")
    sel8 = nc.dram_tensor("sel8", [128, HSH], F32R, kind="ExternalInput")
    sel_d = nc.dram_tensor("sel_d", [D - 1, 128], F32R, kind="ExternalInput")
    sel_f = nc.dram_tensor("sel_f", [128, D - 1], F32R, kind="ExternalInput")
    sel_k = nc.dram_tensor("sel_k", [DK, KK, D], F32R, kind="ExternalInput")
    sel_r = nc.dram_tensor("sel_r", [HSH, 128], F32R, kind="ExternalInput")
    BF16 = mybir.dt.bfloat16
    Wsh = nc.dram_tensor("Wsh", [128, NKC, SSH], BF16, kind="ExternalInput")

    # ---- external outputs (per core) ----
    main_out = nc.dram_tensor("main_out", [H, B, P], F32, kind="ExternalOutput")
    weighted_out = nc.dram_tensor("weighted_out", [HSH, B, P], F32, kind="ExternalOutput")
    attn_out = nc.dram_tensor("attn_out", [B, D - 1], F32, kind="ExternalOutput")
    recon_out = nc.dram_tensor("recon_out", [D, BSH, L], F32, kind="ExternalOutput")

    # ---- internal DRAM scratch ----
    m_d = nc.dram_tensor("m_d", [B, SSH], F32R)
    attnT_d = nc.dram_tensor("attnT_d", [D - 1, B], F32R)
    cc_in = nc.dram_tensor("cc_in", [D - 1, B], F32)
    cc_out = nc.dram_tensor("cc_out", [D - 1, B], F32, addr_space="Shared")

    with tile.TileContext(nc) as tc, ExitStack() as ctx:
        const = ctx.enter_context(tc.tile_pool(name="const", bufs=1))
        big = ctx.enter_context(tc.tile_pool(name="big", bufs=1))
        wpool = ctx.enter_context(tc.tile_pool(name="wpool", bufs=w_bufs))
        ypool = ctx.enter_context(tc.tile_pool(name="ypool", bufs=2))
        yrpool = ctx.enter_context(tc.tile_pool(name="yrpool", bufs=2))
        wtpool = ctx.enter_context(tc.tile_pool(name="wtpool", bufs=2))
        ps_conv = ctx.enter_context(tc.tile_pool(name="ps_conv", bufs=2, space="PSUM"))
        ps_mm = ctx.enter_context(tc.tile_pool(name="ps_mm", bufs=1, space="PSUM"))
        ps_z = ctx.enter_context(tc.tile_pool(name="ps_z", bufs=1, space="PSUM"))
        ps_rec = ctx.enter_context(tc.tile_pool(name="ps_rec", bufs=1, space="PSUM"))
        ps_wt = ctx.enter_context(tc.tile_pool(name="ps_wt", bufs=1, space="PSUM"))

        # ---------- conv-main inputs first: they gate the whole chain ----------
        xrep0 = big.tile([KK, B, T], F32R)
        nc.sync.dma_start(xrep0[:], xrep0_i[:])
        wmain_sb = const.tile([KK, H], F32R)
        nc.sync.dma_start(wmain_sb[:], w_main[:])
        cbm_sb = const.tile([H, 1], F32)
        nc.sync.dma_start(cbm_sb[:], cb_main[:])
        waux_sb = const.tile([DK, 128], F32R)
        nc.sync.dma_start(waux_sb[:], w_aux[:])
        wrec_sb = const.tile([DK, DH], F32R)
        nc.sync.dma_start(wrec_sb[:], w_rec[:])
        wdec_sb = const.tile([128, 8, DK], F32R)
        nc.sync.dma_start(wdec_sb[:], w_dec[:])
        cba_sb = const.tile([128, 1], F32)
        nc.sync.dma_start(cba_sb[:], cb_aux[:])
        cbr_sb = const.tile([128, 8], F32)
        nc.sync.dma_start(cbr_sb[:], cb_rec[:])
        decb_sb = const.tile([D, 1], F32)
        nc.sync.dma_start(decb_sb[:], dec_b[:])
        sel8_sb = const.tile([128, HSH], F32R)
        nc.sync.dma_start(sel8_sb[:], sel8[:])
        seld_sb = const.tile([D - 1, 128], F32R)
        nc.sync.dma_start(seld_sb[:], sel_d[:])
        self_sb = const.tile([128, D - 1], F32R)
        nc.sync.dma_start(self_sb[:], sel_f[:])
        selk_sb = const.tile([DK, KK, D], F32R)
        nc.sync.dma_start(selk_sb[:], sel_k[:])
        selr_sb = const.tile([HSH, 128], F32R)
        nc.sync.dma_start(selr_sb[:], sel_r[:])

        # ---------- im2col inputs (host-prepared, contiguous loads) ----------
        # on scalar so sync can start streaming W immediately
        xrep = big.tile([DK, B, T], F32R)
        nc.scalar.dma_start(xrep[:], xrep_i[:])
        xrrep = big.tile([DK, BSH, T], F32R)
        nc.scalar.dma_start(xrrep[:], xrrep_i[:])

        # ---------- conv main (d=0, all h), chunked over t ----------
        # each chunk completes whole mainT kc-columns, so the big matmul
        # starts as soon as the first chunks land
        pm = big.tile([H, B, P], F32)       # pooled main, (b,p) free order
        mainT_sb = big.tile([128, NKC, B], BF16)
        TC = 16                             # t per conv-main chunk (8 p, 4 kc)
        for c in range(16):                 # 15 full chunks + 1 tail
            tn = TC if c < 15 else T - 15 * TC   # last: 12 t -> 6 p, 3 kc
            pn, kn = tn // 2, tn // 4
            ps = ps_conv.tile([128, B * TC], F32, tag="ps")
            nc.tensor.matmul(ps[:H, 0:B * tn], wmain_sb[:],
                             xrep0[:, :, TC * c:TC * c + tn].transpose([0, 2, 1]),
                             start=True, stop=True)
            ym = ypool.tile([H, TC, B], F32, tag="ym")
            nc.scalar.activation(ym[:, 0:tn, :],
                                 ps[:H, 0:B * tn].rearrange("h (t b) -> h t b", b=B),
                                 ACTF.Relu, bias=cbm_sb[:])
            nc.vector.tensor_tensor(
                pm[:, :, 8 * c:8 * c + pn].transpose([0, 2, 1]),
                ym[:, 0:tn:2, :], ym[:, 1:tn:2, :], op=ALU.add)
            nc.vector.tensor_tensor(mainT_sb[0:64, 4 * c:4 * c + kn, :],
                                    ym[:, 0:tn:4, :], ym[:, 1:tn:4, :], op=ALU.add)
            nc.vector.tensor_tensor(mainT_sb[64:128, 4 * c:4 * c + kn, :],
                                    ym[:, 2:tn:4, :], ym[:, 3:tn:4, :], op=ALU.add)

        # ---------- conv aux (all d, this core's 8 h) ----------
        pa = big.tile([128, B, P], F32R)    # pooled aux [(d,h8), b, p]
        NB = 2                              # batches per aux conv chunk
        for c in range(B // NB):
            ps = ps_conv.tile([128, NB * T], F32, tag="ps")
            nc.tensor.matmul(ps[:], waux_sb[:],
                             xrep[:, NB * c:NB * (c + 1), :],
                             start=True, stop=True)
            ya = ypool.tile([128, NB, T], F32, tag="ya")
            nc.scalar.activation(ya[:], ps[:].rearrange("h (b t) -> h b t", b=NB),
                                 ACTF.Relu, bias=cba_sb[:])
            nc.vector.tensor_tensor(pa[:, NB * c:NB * (c + 1), :],
                                    ya[:, :, 0:T - 1:2], ya[:, :, 1:T:2], op=ALU.add)

        # ---------- big matmul: m = main @ Wsh ----------
        m_ps0 = ps_mm.tile([B, 504], F32, tag="m0")
        m_ps1 = ps_mm.tile([B, 504], F32, tag="m1")
        dma_engines = [nc.sync, nc.gpsimd]
        WMAC = 8                            # k-chunks per W macro-tile DMA
        kc = 0
        ti = 0
        while kc < NKC:
            nmac = min(WMAC, NKC - kc)
            wt = wpool.tile([128, WMAC, SSH], BF16, tag="w")
            dma_engines[ti % 2].dma_start(
                wt[:, 0:nmac, :], Wsh[:, kc:kc + nmac, :])
            for j in range(nmac):
                nc.tensor.matmul(m_ps0[:], mainT_sb[:, kc + j, :],
                                 wt[:, j, 0:504],
                                 start=(kc + j == 0), stop=(kc + j == NKC - 1))
                nc.tensor.matmul(m_ps1[:], mainT_sb[:, kc + j, :],
                                 wt[:, j, 504:SSH],
                                 start=(kc + j == 0), stop=(kc + j == NKC - 1))
            kc += nmac
            ti += 1
        nc.sync.dma_start(main_out[:], pm[:])
        m_sb = big.tile([B, SSH], F32R)
        nc.scalar.copy(m_sb[:, 0:504], m_ps0[:])
        nc.scalar.copy(m_sb[:, 504:SSH], m_ps1[:])

        # ---------- scores partials + AllReduce ----------
        # m8[h8, b, p] = m[b, h8*P + p]; PE-broadcast to all 16 d-slots
        nc.sync.dma_start(m_d[:], m_sb[:])
        m8 = big.tile([HSH, B, P], F32R)
        nc.sync.dma_start(m8[:], m_d[:].rearrange("b (h p) -> h b p", h=HSH))
        m_rep = big.tile([128, B, P], F32)
        m8f = m8[:].rearrange("h b p -> h (b p)")
        for c8 in range(8):
            mr_ps = ps_wt.tile([128, 504], F32, tag="wt")
            nc.tensor.matmul(mr_ps[:], selr_sb[:], m8f[:, 504 * c8:504 * (c8 + 1)],
                             start=True, stop=True)
            nc.scalar.copy(
                m_rep[:].rearrange("q b p -> q (b p)")[:, 504 * c8:504 * (c8 + 1)],
                mr_ps[:])
        nc.vector.tensor_tensor(m_rep[:], m_rep[:], pa[:], op=ALU.mult)
        sc_hb = const.tile([128, B], F32R)
        with nc.allow_low_precision(reason="f32r rounding of fp32-accumulated scores"):
            nc.vector.tensor_reduce(sc_hb[:], m_rep[:], axis=AX.X, op=ALU.add)
        # fold h8 on PE: scT[d', b] = sum_{(d,h8)} sel_f * sc_hb
        sc_ps = ps_wt.tile([D - 1, B], F32, tag="wt")
        nc.tensor.matmul(sc_ps[:], self_sb[:], sc_hb[:], start=True, stop=True)
        scT_sb = const.tile([D - 1, B], F32)
        nc.scalar.copy(scT_sb[:], sc_ps[:])
        nc.sync.dma_start(cc_in[:], scT_sb[:])
        nc.gpsimd.collective_compute(
            "AllReduce", ALU.add, replica_groups=[list(range(NC))],
            ins=[cc_in[:]], outs=[cc_out[:]])

        # ---------- recon path (fills the collective window) ----------
        z_sb = big.tile([DK, BSH, L + 4], F32R)   # zero-padded: z at t+4
        # f32r memset lacks an ISA encoding; zero the margins via ACT scale=0
        nc.scalar.activation(z_sb[:, :, 0:4], selk_sb[:, 0:4, 0:4],
                             ACTF.Copy, scale=0.0)
        nc.scalar.activation(z_sb[:, :, 256:260], selk_sb[:, 0:4, 0:4],
                             ACTF.Copy, scale=0.0)
        xrf = xrrep[:].rearrange("a b t -> a (b t)")
        for h2 in range(2):
            z_ps = ps_z.tile([DK, 504], F32, tag="z")
            for j in range(8):
                ps = ps_conv.tile([128, 504], F32, tag="ps")
                nc.tensor.matmul(ps[:], wrec_sb[:, 128 * j:128 * (j + 1)],
                                 xrf[:, 504 * h2:504 * (h2 + 1)],
                                 start=True, stop=True)
                yr = yrpool.tile([128, 504], F32R, tag="yr")
                nc.scalar.activation(yr[:], ps[:], ACTF.Relu,
                                     bias=cbr_sb[:, j:j + 1])
                nc.tensor.matmul(z_ps[:], wdec_sb[:, j, :], yr[:],
                                 start=(j == 0), stop=(j == 7))
            nc.scalar.copy(z_sb[:, 2 * h2:2 * h2 + 2, 4:4 + T],
                           z_ps[:].rearrange("a (b t) -> a b t", b=2))
        # k-fold via accumulating selector matmuls over shifted input windows
        rec_ps = ps_rec.tile([D, BSH, L], F32)
        for b4 in range(BSH):
            for k in range(KK):
                nc.tensor.matmul(rec_ps[:, b4, :], selk_sb[:, k, :],
                                 z_sb[:, b4, 4 - k:4 - k + L],
                                 start=(k == 0), stop=(k == KK - 1),
                                 skip_group_check=True)
        rec_sb = big.tile([D, BSH, L], F32)
        nc.vector.tensor_scalar_add(rec_sb[:], rec_ps[:], decb_sb[:])
        nc.sync.dma_start(recon_out[:], rec_sb[:])

        # ---------- softmax ----------
        scores = const.tile([B, D - 1], F32)
        nc.sync.dma_start(scores[:], cc_out[:].transpose([1, 0]))
        ex = const.tile([B, D - 1], F32)
        esum = const.tile([B, 1], F32)
        nc.scalar.activation(ex[:], scores[:], ACTF.Exp, accum_out=esum[:])
        rs = const.tile([B, 1], F32)
        nc.vector.reciprocal(rs[:], esum[:])
        attn_sb = const.tile([B, D - 1], F32)
        nc.vector.tensor_scalar_mul(attn_sb[:], ex[:], rs[:])
        nc.sync.dma_start(attn_out[:], attn_sb[:])
        nc.sync.dma_start(attnT_d[:].transpose([1, 0]), attn_sb[:].bitcast(F32R))
        attnT_sb = const.tile([D - 1, B], F32R)
        nc.sync.dma_start(attnT_sb[:], attnT_d[:])

        # ---------- weighted = sum_d attn[b,d] * aux ----------
        arep_ps = ps_conv.tile([128, B], F32, tag="ps")
        nc.tensor.matmul(arep_ps[:], seld_sb[:], attnT_sb[:], start=True, stop=True)
        arep_sb = const.tile([128, B], F32)
        nc.scalar.copy(arep_sb[:], arep_ps[:])
        # wsc = aux * attn (broadcast along p), chunked and pipelined
        NBW = 4                             # batches per selector-matmul chunk
        for c in range(B // NBW):
            nc.vector.tensor_tensor(
                pa[:, NBW * c:NBW * (c + 1), :], pa[:, NBW * c:NBW * (c + 1), :],
                arep_sb[:].unsqueeze(-1).broadcast_to([128, B, P])
                [:, NBW * c:NBW * (c + 1), :], op=ALU.mult)
            wt_ps = ps_wt.tile([HSH, NBW * P], F32, tag="wt")
            nc.tensor.matmul(wt_ps[:], sel8_sb[:],
                             pa[:, NBW * c:NBW * (c + 1), :],
                             start=True, stop=True)
            wchunk = wtpool.tile([HSH, NBW, P], F32, tag="wc")
            nc.scalar.copy(wchunk[:], wt_ps[:].rearrange("h (b p) -> h b p", b=NBW))
            nc.sync.dma_start(weighted_out[:, NBW * c:NBW * (c + 1), :], wchunk[:])

    nc.compile()
    return nc


def _prep_inputs(x, conv_w, conv_b, deconv_w, deconv_b, W_attn):
    """Build the per-core input maps (all numpy float32)."""
    x = np.ascontiguousarray(np.asarray(x, np.float32))
    conv_w = np.asarray(conv_w, np.float32)
    conv_b = np.asarray(conv_b, np.float32)
    deconv_w = np.asarray(deconv_w, np.float32)
    deconv_b = np.asarray(deconv_b, np.float32)
    W_attn = np.asarray(W_attn, np.float32)

    xT = np.ascontiguousarray(x.transpose(2, 0, 1))          # [D, B, L]
    # im2col stacks: xrep[(k,d), b, t] = x[b, t+k, d]
    xrep_i = np.ascontiguousarray(np.concatenate(
        [xT[:, :, k:k + T] for k in range(KK)], axis=0).reshape(DK, B, T))
    xrep0_i = np.ascontiguousarray(
        np.stack([xT[0, :, k:k + T] for k in range(KK)], axis=0))
    # rows permuted: s' = 64p + h  <->  s = 126h + p
    W_perm = np.ascontiguousarray(
        W_attn.reshape(H, P, S).transpose(1, 0, 2).reshape(S, S))

    # conv stationaries, (k,d) partition order; aux/main scaled by 0.5
    kd = np.zeros((DK, D, H), np.float32)                    # [(k,d), d2, h]
    for k in range(KK):
        for d in range(D):
            kd[16 * k + d, d, :] = conv_w[d, :, k]
    w_main = np.ascontiguousarray(0.5 * conv_w[0].T)          # [5, 64]
    w_rec = np.ascontiguousarray(kd.reshape(DK, DH))          # [80, 1024]

    wdec = np.zeros((DH, DK), np.float32)
    for d in range(D):
        for k in range(KK):
            wdec[d * H:(d + 1) * H, 16 * k + d] = deconv_w[d, :, k]
    w_dec = np.ascontiguousarray(wdec.reshape(8, 128, DK).transpose(1, 0, 2))

    cb_main = np.ascontiguousarray(0.5 * conv_b[0, :, None])  # [64,1]
    cb_rec = np.ascontiguousarray(conv_b.reshape(DH).reshape(8, 128).T)
    dec_b = np.ascontiguousarray(deconv_b[:, None])           # [16,1]

    # sel8[(d,h8), h8'] = [h8==h8'] for d>=1 else 0 -> [128, 8]
    sel8 = np.ascontiguousarray(np.concatenate(
        [np.zeros((HSH, HSH), np.float32),
         np.tile(np.eye(HSH, dtype=np.float32), (D - 1, 1))], axis=0))
    # sel_d[d', (d,h8)] = [d==d'+1] -> [15, 128]
    sel_d = np.zeros((D - 1, 128), np.float32)
    for dp in range(D - 1):
        sel_d[dp, (dp + 1) * HSH:(dp + 2) * HSH] = 1.0
    # sel_f[(d,h8), d'] = [d==d'+1] -> [128, 15]
    sel_f = np.ascontiguousarray(sel_d.T)
    # sel_r[h8', (d,h8)] = [h8==h8'] -> [8, 128] (PE broadcast of m8)
    sel_r = np.ascontiguousarray(np.tile(np.eye(HSH, dtype=np.float32), (1, D)))
    # sel_k[(k',d'), k, d] = [k'==k][d'==d] -> [80, 5, 16]
    sel_k = np.zeros((DK, KK, D), np.float32)
    for k in range(KK):
        for d in range(D):
            sel_k[16 * k + d, k, d] = 1.0

    in_maps = []
    for c in range(NC):
        wa = np.zeros((DK, 128), np.float32)
        for k in range(KK):
            for d in range(D):
                wa[16 * k + d, d * HSH:(d + 1) * HSH] = \
                    0.5 * conv_w[d, HSH * c:HSH * (c + 1), k]
        cb_aux = np.ascontiguousarray(
            0.5 * conv_b[:, HSH * c:HSH * (c + 1)].reshape(128)[:, None])
        xslice = xT[:, BSH * c:BSH * (c + 1), :]
        xrrep_i = np.ascontiguousarray(np.concatenate(
            [xslice[:, :, k:k + T] for k in range(KK)], axis=0).reshape(DK, BSH, T))
        in_maps.append({
            "xrep_i": xrep_i,
            "xrep0_i": xrep0_i,
            "xrrep_i": xrrep_i,
            "w_aux": wa,
            "w_main": w_main,
            "w_rec": w_rec,
            "w_dec": w_dec,
            "cb_aux": cb_aux,
            "cb_main": cb_main,
            "cb_rec": cb_rec,
            "dec_b": dec_b,
            "sel8": sel8,
            "sel_d": sel_d,
            "sel_f": sel_f,
            "sel_k": sel_k,
            "sel_r": sel_r,
            "Wsh": np.ascontiguousarray(
                W_perm[:, SSH * c:SSH * (c + 1)].astype(ml_dtypes.bfloat16)
                .reshape(NKC, 128, SSH).transpose(1, 0, 2)),
        })
    return in_maps


def _assemble(results):
    main = np.ascontiguousarray(
        results[0]["main_out"].transpose(1, 0, 2)).reshape(B, S)
    weighted = np.empty((B, H, P), np.float32)
    for c, r in enumerate(results):
        weighted[:, HSH * c:HSH * (c + 1), :] = r["weighted_out"].transpose(1, 0, 2)
    weighted = weighted.reshape(B, S)
    attn = results[0]["attn_out"][:, None, :]                  # [B,1,15]
    recon = np.concatenate(
        [r["recon_out"].transpose(1, 0, 2) for r in results], axis=0)
    out = np.concatenate([main[:, None, :], weighted[:, None, :]], axis=2)
    return out, attn, recon


def kernel(x, conv_w, conv_b, deconv_w, deconv_b, W_attn,
           _trace=False, _trace_kwargs=None):
    if "nc" not in _CACHED:
        _CACHED["nc"] = build_nc()
    nc = _CACHED["nc"]
    in_maps = _prep_inputs(x, conv_w, conv_b, deconv_w, deconv_b, W_attn)
    res = run_bass_kernel_spmd(nc, in_maps, list(range(NC)), trace=_trace,
                               **(_trace_kwargs or {}))
    outs = _assemble(res.results)
    if _trace:
        return outs, res
    return outs


if __name__ == "__main__":
    d = np.load("/root/problem/ref_cache.npz")
    out, attn, recon = kernel(d["x"], d["conv_w"], d["conv_b"],
                              d["deconv_w"], d["deconv_b"], d["W_attn"])
    for name, a, b in [("out", out, d["out"]), ("attn", attn, d["attn"]),
                       ("recon", recon, d["recon"])]:
        err = np.abs(a - b).max() / (np.abs(b).max() + 1e-30)
        print(f"{name}: rel_err {err:.3e}")


# revision 40
# speedup vs baseline: 1.0841x; 1.0087x over previous
"""Trainium2 Bass kernel for nn_Encoder (dense_cnn).

Full-input contract: kernel(**inputs) takes the unsharded numpy inputs and
returns (out [B,1,2S], attn [B,1,D-1], recon [B,D,L]) matching the fp32
reference.

Sharding across 8 NeuronCores:
  - W_attn [S,S] is sharded column-wise: core c gets columns [1008c, 1008c+1008),
    with rows permuted to s' = 64p + h so the lhsT (mainT) tiles are buildable
    from the pooled conv output with two strided DVE copies (no transpose DMA).
  - Convs are computed where needed: each core computes main (d=0, all h)
    fully, the aux slice for its h-range [8c, 8c+8) x all d, and the recon
    path for its 4 batches [4c, 4c+4).
  - Partial attention scores are folded to [15,32] on PE and AllReduced;
    softmax is replicated; each core emits its weighted slice [B, 1008].
All matmuls run as float32r (full PE rate at N>=256, tf32-class mantissa).
"""

import numpy as np
import ml_dtypes
from contextlib import ExitStack

import concourse.bass as bass
import concourse.bacc as bacc
import concourse.tile as tile
import concourse.mybir as mybir
from concourse.bass_utils import run_bass_kernel_spmd

F32 = mybir.dt.float32
F32R = mybir.dt.float32r
AX = mybir.AxisListType
ALU = mybir.AluOpType
ACTF = mybir.ActivationFunctionType

B, L, D, H, KK = 32, 256, 16, 64, 5
T = L - KK + 1          # 252
P = T // 2              # 126
S = P * H               # 8064
NC = 8                  # cores
HSH = H // NC           # 8 h-values per core
SSH = S // NC           # 1008 columns of W / weighted per core
BSH = B // NC           # 4 batches per core for recon
DK = D * KK             # 80  (k,d) partitions for conv im2col
DH = D * H              # 1024 (d,h) partitions for recon conv
NKC = S // 128          # 63 contraction chunks for the big matmul

_CACHED = {}


def build_nc(w_bufs=2):
    nc = bacc.Bacc(None, num_devices=NC)

    # ---- external inputs (per core) ----
    xrep_i = nc.dram_tensor("xrep_i", [DK, B, T], F32R, kind="ExternalInput")
    xrep0_i = nc.dram_tensor("xrep0_i", [KK, B, T], F32R, kind="ExternalInput")
    xrrep_i = nc.dram_tensor("xrrep_i", [DK, BSH, T], F32R, kind="ExternalInput")
    w_aux = nc.dram_tensor("w_aux", [DK, 128], F32R, kind="ExternalInput")
    w_main = nc.dram_tensor("w_main", [KK, H], F32R, kind="ExternalInput")
    w_rec = nc.dram_tensor("w_rec", [DK, DH], F32R, kind="ExternalInput")
    w_dec = nc.dram_tensor("w_dec", [128, 8, DK], F32R, kind="ExternalInput")
    cb_aux = nc.dram_tensor("cb_aux", [128, 1], F32, kind="ExternalInput")
    cb_main = nc.dram_tensor("cb_main", [H, 1], F32, kind="ExternalInput")
    cb_rec = nc.dram_tensor("cb_rec", [128, 8], F32, kind="ExternalInput")
    dec_b = nc.dram_tensor("dec_b", [D, 1], F32, kind="ExternalInput")
    sel8 = nc.dram_tensor("sel8", [128, HSH], F32R, kind="ExternalInput")
    sel_d = nc.dram_tensor("sel_d", [D - 1, 128], F32R, kind="ExternalInput")
    sel_f = nc.dram_tensor("sel_f", [128, D - 1], F32R, kind="ExternalInput")
    sel_k = nc.dram_tensor("sel_k", [DK, KK, D], F32R, kind="ExternalInput")
    sel_r = nc.dram_tensor("sel_r", [HSH, 128], F32R, kind="ExternalInput")
    BF16 = mybir.dt.bfloat16
    Wsh = nc.dram_tensor("Wsh", [128, NKC, SSH], BF16, kind="ExternalInput")

    # ---- external outputs (per core) ----
    main_out = nc.dram_tensor("main_out", [H, B, P], F32, kind="ExternalOutput")
    weighted_out = nc.dram_tensor("weighted_out", [HSH, B, P], F32, kind="ExternalOutput")
    attn_out = nc.dram_tensor("attn_out", [B, D - 1], F32, kind="ExternalOutput")
    recon_out = nc.dram_tensor("recon_out", [D, BSH, L], F32, kind="ExternalOutput")

    # ---- internal DRAM scratch ----
    m_d = nc.dram_tensor("m_d", [B, SSH], F32R)
    attnT_d = nc.dram_tensor("attnT_d", [D - 1, B], F32R)
    cc_in = nc.dram_tensor("cc_in", [D - 1, B], F32)
    cc_out = nc.dram_tensor("cc_out", [D - 1, B], F32, addr_space="Shared")

    with tile.TileContext(nc) as tc, ExitStack() as ctx:
        const = ctx.enter_context(tc.tile_pool(name="const", bufs=1))
        big = ctx.enter_context(tc.tile_pool(name="big", bufs=1))
        wpool = ctx.enter_context(tc.tile_pool(name="wpool", bufs=w_bufs))
        ypool = ctx.enter_context(tc.tile_pool(name="ypool", bufs=2))
        yrpool = ctx.enter_context(tc.tile_pool(name="yrpool", bufs=2))
        wtpool = ctx.enter_context(tc.tile_pool(name="wtpool", bufs=2))
        ps_conv = ctx.enter_context(tc.tile_pool(name="ps_conv", bufs=2, space="PSUM"))
        ps_mm = ctx.enter_context(tc.tile_pool(name="ps_mm", bufs=1, space="PSUM"))
        ps_z = ctx.enter_context(tc.tile_pool(name="ps_z", bufs=1, space="PSUM"))
        ps_rec = ctx.enter_context(tc.tile_pool(name="ps_rec", bufs=1, space="PSUM"))
        ps_wt = ctx.enter_context(tc.tile_pool(name="ps_wt", bufs=1, space="PSUM"))

        # ---------- conv-main inputs first: they gate the whole chain ----------
        xrep0 = big.tile([KK, B, T], F32R)
        nc.sync.dma_start(xrep0[:], xrep0_i[:])
        wmain_sb = const.tile([KK, H], F32R)
        nc.sync.dma_start(wmain_sb[:], w_main[:])
        cbm_sb = const.tile([H, 1], F32)
        nc.sync.dma_start(cbm_sb[:], cb_main[:])
        waux_sb = const.tile([DK, 128], F32R)
        nc.sync.dma_start(waux_sb[:], w_aux[:])
        wrec_sb = const.tile([DK, DH], F32R)
        nc.sync.dma_start(wrec_sb[:], w_rec[:])
        wdec_sb = const.tile([128, 8, DK], F32R)
        nc.sync.dma_start(wdec_sb[:], w_dec[:])
        cba_sb = const.tile([128, 1], F32)
        nc.sync.dma_start(cba_sb[:], cb_aux[:])
        cbr_sb = const.tile([128, 8], F32)
        nc.sync.dma_start(cbr_sb[:], cb_rec[:])
        decb_sb = const.tile([D, 1], F32)
        nc.sync.dma_start(decb_sb[:], dec_b[:])
        sel8_sb = const.tile([128, HSH], F32R)
        nc.sync.dma_start(sel8_sb[:], sel8[:])
        seld_sb = const.tile([D - 1, 128], F32R)
        nc.sync.dma_start(seld_sb[:], sel_d[:])
        self_sb = const.tile([128, D - 1], F32R)
        nc.sync.dma_start(self_sb[:], sel_f[:])
        selk_sb = const.tile([DK, KK, D], F32R)
        nc.sync.dma_start(selk_sb[:], sel_k[:])
        selr_sb = const.tile([HSH, 128], F32R)
        nc.sync.dma_start(selr_sb[:], sel_r[:])

        # ---------- im2col inputs (host-prepared, contiguous loads) ----------
        # on scalar so sync can start streaming W immediately
        xrep = big.tile([DK, B, T], F32R)
        nc.gpsimd.dma_start(xrep[:], xrep_i[:])
        xrrep = big.tile([DK, BSH, T], F32R)
        nc.gpsimd.dma_start(xrrep[:], xrrep_i[:])

        # ---------- conv main (d=0, all h), chunked over t ----------
        # each chunk completes whole mainT kc-columns, so the big matmul
        # starts as soon as the first chunks land
        pm = big.tile([H, B, P], F32)       # pooled main, (b,p) free order
        mainT_sb = big.tile([128, NKC, B], BF16)
        TC = 16                             # t per conv-main chunk (8 p, 4 kc)
        for c in range(16):                 # 15 full chunks + 1 tail
            tn = TC if c < 15 else T - 15 * TC   # last: 12 t -> 6 p, 3 kc
            pn, kn = tn // 2, tn // 4
            ps = ps_conv.tile([128, B * TC], F32, tag="ps")
            nc.tensor.matmul(ps[:H, 0:B * tn], wmain_sb[:],
                             xrep0[:, :, TC * c:TC * c + tn].transpose([0, 2, 1]),
                             start=True, stop=True)
            ym = ypool.tile([H, TC, B], F32, tag="ym")
            nc.scalar.activation(ym[:, 0:tn, :],
                                 ps[:H, 0:B * tn].rearrange("h (t b) -> h t b", b=B),
                                 ACTF.Relu, bias=cbm_sb[:])
            nc.vector.tensor_tensor(
                pm[:, :, 8 * c:8 * c + pn].transpose([0, 2, 1]),
                ym[:, 0:tn:2, :], ym[:, 1:tn:2, :], op=ALU.add)
            nc.vector.tensor_tensor(mainT_sb[0:64, 4 * c:4 * c + kn, :],
                                    ym[:, 0:tn:4, :], ym[:, 1:tn:4, :], op=ALU.add)
            nc.vector.tensor_tensor(mainT_sb[64:128, 4 * c:4 * c + kn, :],
                                    ym[:, 2:tn:4, :], ym[:, 3:tn:4, :], op=ALU.add)

        # ---------- conv aux (all d, this core's 8 h) ----------
        pa = big.tile([128, B, P], F32R)    # pooled aux [(d,h8), b, p]
        NB = 2                              # batches per aux conv chunk
        for c in range(B // NB):
            ps = ps_conv.tile([128, NB * T], F32, tag="ps")
            nc.tensor.matmul(ps[:], waux_sb[:],
                             xrep[:, NB * c:NB * (c + 1), :],
                             start=True, stop=True)
            ya = ypool.tile([128, NB, T], F32, tag="ya")
            nc.scalar.activation(ya[:], ps[:].rearrange("h (b t) -> h b t", b=NB),
                                 ACTF.Relu, bias=cba_sb[:])
            nc.vector.tensor_tensor(pa[:, NB * c:NB * (c + 1), :],
                                    ya[:, :, 0:T - 1:2], ya[:, :, 1:T:2], op=ALU.add)

        # ---------- big matmul: m = main @ Wsh ----------
        m_ps0 = ps_mm.tile([B, 504], F32, tag="m0")
        m_ps1 = ps_mm.tile([B, 504], F32, tag="m1")
        dma_engines = [nc.sync, nc.scalar]
        WMAC = 8                            # k-chunks per W macro-tile DMA
        kc = 0
        ti = 0
        while kc < NKC:
            nmac = min(WMAC, NKC - kc)
            wt = wpool.tile([128, WMAC, SSH], BF16, tag="w")
            dma_engines[ti % 2].dma_start(
                wt[:, 0:nmac, :], Wsh[:, kc:kc + nmac, :])
            for j in range(nmac):
                nc.tensor.matmul(m_ps0[:], mainT_sb[:, kc + j, :],
                                 wt[:, j, 0:504],
                                 start=(kc + j == 0), stop=(kc + j == NKC - 1))
                nc.tensor.matmul(m_ps1[:], mainT_sb[:, kc + j, :],
                                 wt[:, j, 504:SSH],
                                 start=(kc + j == 0), stop=(kc + j == NKC - 1))
            kc += nmac
            ti += 1
        nc.sync.dma_start(main_out[:], pm[:])
        m_sb = big.tile([B, SSH], F32R)
        nc.scalar.copy(m_sb[:, 0:504], m_ps0[:])
        nc.scalar.copy(m_sb[:, 504:SSH], m_ps1[:])

        # ---------- scores partials + AllReduce ----------
        # m8[h8, b, p] = m[b, h8*P + p]; PE-broadcast to all 16 d-slots
        nc.sync.dma_start(m_d[:], m_sb[:])
        m8 = big.tile([HSH, B, P], F32R)
        nc.sync.dma_start(m8[:], m_d[:].rearrange("b (h p) -> h b p", h=HSH))
        m_rep = big.tile([128, B, P], F32)
        sc_hb = const.tile([128, B], F32R)
        m8f = m8[:].rearrange("h b p -> h (b p)")
        mrf = m_rep[:].rearrange("q b p -> q (b p)")
        paf = pa[:].rearrange("q b p -> q (b p)")
        for c8 in range(8):
            cs = slice(504 * c8, 504 * (c8 + 1))
            mr_ps = ps_wt.tile([128, 504], F32, tag="wt")
            nc.tensor.matmul(mr_ps[:], selr_sb[:], m8f[:, cs],
                             start=True, stop=True)
            nc.scalar.copy(mrf[:, cs], mr_ps[:])
            nc.vector.tensor_tensor(mrf[:, cs], mrf[:, cs], paf[:, cs], op=ALU.mult)
            with nc.allow_low_precision(reason="f32r scores rounding"):
                nc.vector.tensor_reduce(
                    sc_hb[:, 4 * c8:4 * (c8 + 1)],
                    m_rep[:, 4 * c8:4 * (c8 + 1), :], axis=AX.X, op=ALU.add)
        # fold h8 on PE: scT[d', b] = sum_{(d,h8)} sel_f * sc_hb
        sc_ps = ps_wt.tile([D - 1, B], F32, tag="wt")
        nc.tensor.matmul(sc_ps[:], self_sb[:], sc_hb[:], start=True, stop=True)
        scT_sb = const.tile([D - 1, B], F32)
        nc.scalar.copy(scT_sb[:], sc_ps[:])
        nc.sync.dma_start(cc_in[:], scT_sb[:])
        nc.gpsimd.collective_compute(
            "AllReduce", ALU.add, replica_groups=[list(range(NC))],
            ins=[cc_in[:]], outs=[cc_out[:]])

        # ---------- recon path (fills the collective window) ----------
        z_sb = big.tile([DK, BSH, L + 4], F32R)   # zero-padded: z at t+4
        # f32r memset lacks an ISA encoding; zero the margins via ACT scale=0
        nc.scalar.activation(z_sb[:, :, 0:4], selk_sb[:, 0:4, 0:4],
                             ACTF.Copy, scale=0.0)
        nc.scalar.activation(z_sb[:, :, 256:260], selk_sb[:, 0:4, 0:4],
                             ACTF.Copy, scale=0.0)
        xrf = xrrep[:].rearrange("a b t -> a (b t)")
        for h2 in range(2):
            z_ps = ps_z.tile([DK, 504], F32, tag="z")
            for j in range(8):
                ps = ps_conv.tile([128, 504], F32, tag="ps")
                nc.tensor.matmul(ps[:], wrec_sb[:, 128 * j:128 * (j + 1)],
                                 xrf[:, 504 * h2:504 * (h2 + 1)],
                                 start=True, stop=True)
                yr = yrpool.tile([128, 504], F32R, tag="yr")
                nc.scalar.activation(yr[:], ps[:], ACTF.Relu,
                                     bias=cbr_sb[:, j:j + 1])
                nc.tensor.matmul(z_ps[:], wdec_sb[:, j, :], yr[:],
                                 start=(j == 0), stop=(j == 7))
            nc.scalar.copy(z_sb[:, 2 * h2:2 * h2 + 2, 4:4 + T],
                           z_ps[:].rearrange("a (b t) -> a b t", b=2))
        # k-fold via accumulating selector matmuls over shifted input windows
        rec_ps = ps_rec.tile([D, BSH, L], F32)
        for b4 in range(BSH):
            for k in range(KK):
                nc.tensor.matmul(rec_ps[:, b4, :], selk_sb[:, k, :],
                                 z_sb[:, b4, 4 - k:4 - k + L],
                                 start=(k == 0), stop=(k == KK - 1),
                                 skip_group_check=True)
        rec_sb = big.tile([D, BSH, L], F32)
        nc.vector.tensor_scalar_add(rec_sb[:], rec_ps[:], decb_sb[:])
        nc.sync.dma_start(recon_out[:], rec_sb[:])

        # ---------- softmax ----------
        scores = const.tile([B, D - 1], F32)
        nc.sync.dma_start(scores[:], cc_out[:].transpose([1, 0]))
        ex = const.tile([B, D - 1], F32)
        esum = const.tile([B, 1], F32)
        nc.scalar.activation(ex[:], scores[:], ACTF.Exp, accum_out=esum[:])
        rs = const.tile([B, 1], F32)
        nc.vector.reciprocal(rs[:], esum[:])
        attn_sb = const.tile([B, D - 1], F32)
        nc.vector.tensor_scalar_mul(attn_sb[:], ex[:], rs[:])
        nc.sync.dma_start(attn_out[:], attn_sb[:])
        nc.sync.dma_start(attnT_d[:].transpose([1, 0]), attn_sb[:].bitcast(F32R))
        attnT_sb = const.tile([D - 1, B], F32R)
        nc.sync.dma_start(attnT_sb[:], attnT_d[:])

        # ---------- weighted = sum_d attn[b,d] * aux ----------
        arep_ps = ps_conv.tile([128, B], F32, tag="ps")
        nc.tensor.matmul(arep_ps[:], seld_sb[:], attnT_sb[:], start=True, stop=True)
        arep_sb = const.tile([128, B], F32)
        nc.scalar.copy(arep_sb[:], arep_ps[:])
        # wsc = aux * attn (broadcast along p), chunked and pipelined
        NBW = 4                             # batches per selector-matmul chunk
        for c in range(B // NBW):
            nc.vector.tensor_tensor(
                pa[:, NBW * c:NBW * (c + 1), :], pa[:, NBW * c:NBW * (c + 1), :],
                arep_sb[:].unsqueeze(-1).broadcast_to([128, B, P])
                [:, NBW * c:NBW * (c + 1), :], op=ALU.mult)
            wt_ps = ps_wt.tile([HSH, NBW * P], F32, tag="wt")
            nc.tensor.matmul(wt_ps[:], sel8_sb[:],
                             pa[:, NBW * c:NBW * (c + 1), :],
                             start=True, stop=True)
            wchunk = wtpool.tile([HSH, NBW, P], F32, tag="wc")
            nc.scalar.copy(wchunk[:], wt_ps[:].rearrange("h (b p) -> h b p", b=NBW))
            nc.sync.dma_start(weighted_out[:, NBW * c:NBW * (c + 1), :], wchunk[:])

    nc.compile()
    return nc


def _prep_inputs(x, conv_w, conv_b, deconv_w, deconv_b, W_attn):
    """Build the per-core input maps (all numpy float32)."""
    x = np.ascontiguousarray(np.asarray(x, np.float32))
    conv_w = np.asarray(conv_w, np.float32)
    conv_b = np.asarray(conv_b, np.float32)
    deconv_w = np.asarray(deconv_w, np.float32)
    deconv_b = np.asarray(deconv_b, np.float32)
    W_attn = np.asarray(W_attn, np.float32)

    xT = np.ascontiguousarray(x.transpose(2, 0, 1))          # [D, B, L]
    # im2col stacks: xrep[(k,d), b, t] = x[b, t+k, d]
    xrep_i = np.ascontiguousarray(np.concatenate(
        [xT[:, :, k:k + T] for k in range(KK)], axis=0).reshape(DK, B, T))
    xrep0_i = np.ascontiguousarray(
        np.stack([xT[0, :, k:k + T] for k in range(KK)], axis=0))
    # rows permuted: s' = 64p + h  <->  s = 126h + p
    W_perm = np.ascontiguousarray(
        W_attn.reshape(H, P, S).transpose(1, 0, 2).reshape(S, S))

    # conv stationaries, (k,d) partition order; aux/main scaled by 0.5
    kd = np.zeros((DK, D, H), np.float32)                    # [(k,d), d2, h]
    for k in range(KK):
        for d in range(D):
            kd[16 * k + d, d, :] = conv_w[d, :, k]
    w_main = np.ascontiguousarray(0.5 * conv_w[0].T)          # [5, 64]
    w_rec = np.ascontiguousarray(kd.reshape(DK, DH))          # [80, 1024]

    wdec = np.zeros((DH, DK), np.float32)
    for d in range(D):
        for k in range(KK):
            wdec[d * H:(d + 1) * H, 16 * k + d] = deconv_w[d, :, k]
    w_dec = np.ascontiguousarray(wdec.reshape(8, 128, DK).transpose(1, 0, 2))

    cb_main = np.ascontiguousarray(0.5 * conv_b[0, :, None])  # [64,1]
    cb_rec = np.ascontiguousarray(conv_b.reshape(DH).reshape(8, 128).T)
    dec_b = np.ascontiguousarray(deconv_b[:, None])           # [16,1]

    # sel8[(d,h8), h8'] = [h8==h8'] for d>=1 else 0 -> [128, 8]
    sel8 = np.ascontiguousarray(np.concatenate(
        [np.zeros((HSH, HSH), np.float32),
         np.tile(np.eye(HSH, dtype=np.float32), (D - 1, 1))], axis=0))
    # sel_d[d', (d,h8)] = [d==d'+1] -> [15, 128]
    sel_d = np.zeros((D - 1, 128), np.float32)
    for dp in range(D - 1):
        sel_d[dp, (dp + 1) * HSH:(dp + 2) * HSH] = 1.0
    # sel_f[(d,h8), d'] = [d==d'+1] -> [128, 15]
    sel_f = np.ascontiguousarray(sel_d.T)
    # sel_r[h8', (d,h8)] = [h8==h8'] -> [8, 128] (PE broadcast of m8)
    sel_r = np.ascontiguousarray(np.tile(np.eye(HSH, dtype=np.float32), (1, D)))
    # sel_k[(k',d'), k, d] = [k'==k][d'==d] -> [80, 5, 16]
    sel_k = np.zeros((DK, KK, D), np.float32)
    for k in range(KK):
        for d in range(D):
            sel_k[16 * k + d, k, d] = 1.0

    in_maps = []
    for c in range(NC):
        wa = np.zeros((DK, 128), np.float32)
        for k in range(KK):
            for d in range(D):
                wa[16 * k + d, d * HSH:(d + 1) * HSH] = \
                    0.5 * conv_w[d, HSH * c:HSH * (c + 1), k]
        cb_aux = np.ascontiguousarray(
            0.5 * conv_b[:, HSH * c:HSH * (c + 1)].reshape(128)[:, None])
        xslice = xT[:, BSH * c:BSH * (c + 1), :]
        xrrep_i = np.ascontiguousarray(np.concatenate(
            [xslice[:, :, k:k + T] for k in range(KK)], axis=0).reshape(DK, BSH, T))
        in_maps.append({
            "xrep_i": xrep_i,
            "xrep0_i": xrep0_i,
            "xrrep_i": xrrep_i,
            "w_aux": wa,
            "w_main": w_main,
            "w_rec": w_rec,
            "w_dec": w_dec,
            "cb_aux": cb_aux,
            "cb_main": cb_main,
            "cb_rec": cb_rec,
            "dec_b": dec_b,
            "sel8": sel8,
            "sel_d": sel_d,
            "sel_f": sel_f,
            "sel_k": sel_k,
            "sel_r": sel_r,
            "Wsh": np.ascontiguousarray(
                W_perm[:, SSH * c:SSH * (c + 1)].astype(ml_dtypes.bfloat16)
                .reshape(NKC, 128, SSH).transpose(1, 0, 2)),
        })
    return in_maps


def _assemble(results):
    main = np.ascontiguousarray(
        results[0]["main_out"].transpose(1, 0, 2)).reshape(B, S)
    weighted = np.empty((B, H, P), np.float32)
    for c, r in enumerate(results):
        weighted[:, HSH * c:HSH * (c + 1), :] = r["weighted_out"].transpose(1, 0, 2)
    weighted = weighted.reshape(B, S)
    attn = results[0]["attn_out"][:, None, :]                  # [B,1,15]
    recon = np.concatenate(
        [r["recon_out"].transpose(1, 0, 2) for r in results], axis=0)
    out = np.concatenate([main[:, None, :], weighted[:, None, :]], axis=2)
    return out, attn, recon


def kernel(x, conv_w, conv_b, deconv_w, deconv_b, W_attn,
           _trace=False, _trace_kwargs=None):
    if "nc" not in _CACHED:
        _CACHED["nc"] = build_nc()
    nc = _CACHED["nc"]
    in_maps = _prep_inputs(x, conv_w, conv_b, deconv_w, deconv_b, W_attn)
    res = run_bass_kernel_spmd(nc, in_maps, list(range(NC)), trace=_trace,
                               **(_trace_kwargs or {}))
    outs = _assemble(res.results)
    if _trace:
        return outs, res
    return outs


if __name__ == "__main__":
    d = np.load("/root/problem/ref_cache.npz")
    out, attn, recon = kernel(d["x"], d["conv_w"], d["conv_b"],
                              d["deconv_w"], d["deconv_b"], d["W_attn"])
    for name, a, b in [("out", out, d["out"]), ("attn", attn, d["attn"]),
                       ("recon", recon, d["recon"])]:
        err = np.abs(a - b).max() / (np.abs(b).max() + 1e-30)
        print(f"{name}: rel_err {err:.3e}")


# revision 41
# speedup vs baseline: 1.1821x; 1.0904x over previous
"""Trainium2 Bass kernel for nn_Encoder (dense_cnn).

Full-input contract: kernel(**inputs) takes the unsharded numpy inputs and
returns (out [B,1,2S], attn [B,1,D-1], recon [B,D,L]) matching the fp32
reference.

Sharding across 8 NeuronCores:
  - W_attn [S,S] is sharded column-wise: core c gets columns [1008c, 1008c+1008),
    with rows permuted to s' = 64p + h so the lhsT (mainT) tiles are buildable
    from the pooled conv output with two strided DVE copies (no transpose DMA).
  - Convs are computed where needed: each core computes main (d=0, all h)
    fully, the aux slice for its h-range [8c, 8c+8) x all d, and the recon
    path for its 4 batches [4c, 4c+4).
  - Partial attention scores are folded to [15,32] on PE and AllReduced;
    softmax is replicated; each core emits its weighted slice [B, 1008].
All matmuls run as float32r (full PE rate at N>=256, tf32-class mantissa).
"""

import numpy as np
import ml_dtypes
from contextlib import ExitStack

import concourse.bass as bass
import concourse.bacc as bacc
import concourse.tile as tile
import concourse.mybir as mybir
from concourse.bass_utils import run_bass_kernel_spmd

F32 = mybir.dt.float32
F32R = mybir.dt.float32r
AX = mybir.AxisListType
ALU = mybir.AluOpType
ACTF = mybir.ActivationFunctionType

B, L, D, H, KK = 32, 256, 16, 64, 5
T = L - KK + 1          # 252
P = T // 2              # 126
S = P * H               # 8064
NC = 8                  # cores
HSH = H // NC           # 8 h-values per core
SSH = S // NC           # 1008 columns of W / weighted per core
BSH = B // NC           # 4 batches per core for recon
DK = D * KK             # 80  (k,d) partitions for conv im2col
DH = D * H              # 1024 (d,h) partitions for recon conv
NKC = S // 128          # 63 contraction chunks for the big matmul

_CACHED = {}


def build_nc(w_bufs=2):
    nc = bacc.Bacc(None, num_devices=NC)

    # ---- external inputs (per core) ----
    xrep_i = nc.dram_tensor("xrep_i", [DK, B, T], F32R, kind="ExternalInput")
    xrep0_i = nc.dram_tensor("xrep0_i", [KK, B, T], F32R, kind="ExternalInput")
    xrrep_i = nc.dram_tensor("xrrep_i", [DK, BSH, T], F32R, kind="ExternalInput")
    w_aux = nc.dram_tensor("w_aux", [DK, 128], F32R, kind="ExternalInput")
    w_main = nc.dram_tensor("w_main", [KK, H], F32R, kind="ExternalInput")
    w_rec = nc.dram_tensor("w_rec", [DK, DH], F32R, kind="ExternalInput")
    w_dec = nc.dram_tensor("w_dec", [128, 8, DK], F32R, kind="ExternalInput")
    cb_aux = nc.dram_tensor("cb_aux", [128, 1], F32, kind="ExternalInput")
    cb_main = nc.dram_tensor("cb_main", [H, 1], F32, kind="ExternalInput")
    cb_rec = nc.dram_tensor("cb_rec", [128, 8], F32, kind="ExternalInput")
    dec_b = nc.dram_tensor("dec_b", [D, 1], F32, kind="ExternalInput")
    sel8 = nc.dram_tensor("sel8", [128, HSH], F32R, kind="ExternalInput")
    sel_d = nc.dram_tensor("sel_d", [D - 1, 128], F32R, kind="ExternalInput")
    sel_f = nc.dram_tensor("sel_f", [128, D - 1], F32R, kind="ExternalInput")
    sel_k = nc.dram_tensor("sel_k", [DK, KK, D], F32R, kind="ExternalInput")
    sel_r = nc.dram_tensor("sel_r", [HSH, 128], F32R, kind="ExternalInput")
    BF16 = mybir.dt.bfloat16
    Wsh = nc.dram_tensor("Wsh", [128, NKC, SSH], BF16, kind="ExternalInput")

    # ---- external outputs (per core) ----
    main_out = nc.dram_tensor("main_out", [H, B, P], F32, kind="ExternalOutput")
    weighted_out = nc.dram_tensor("weighted_out", [HSH, B, P], F32, kind="ExternalOutput")
    attn_out = nc.dram_tensor("attn_out", [B, D - 1], F32, kind="ExternalOutput")
    recon_out = nc.dram_tensor("recon_out", [D, BSH, L], F32, kind="ExternalOutput")

    # ---- internal DRAM scratch ----
    m_d = nc.dram_tensor("m_d", [B, SSH], F32R)
    attnT_d = nc.dram_tensor("attnT_d", [D - 1, B], F32R)
    cc_in = nc.dram_tensor("cc_in", [D - 1, B], F32)
    cc_out = nc.dram_tensor("cc_out", [D - 1, B], F32, addr_space="Shared")

    with tile.TileContext(nc) as tc, ExitStack() as ctx:
        const = ctx.enter_context(tc.tile_pool(name="const", bufs=1))
        big = ctx.enter_context(tc.tile_pool(name="big", bufs=1))
        wpool = ctx.enter_context(tc.tile_pool(name="wpool", bufs=w_bufs))
        ypool = ctx.enter_context(tc.tile_pool(name="ypool", bufs=2))
        yrpool = ctx.enter_context(tc.tile_pool(name="yrpool", bufs=2))
        wtpool = ctx.enter_context(tc.tile_pool(name="wtpool", bufs=2))
        ps_conv = ctx.enter_context(tc.tile_pool(name="ps_conv", bufs=2, space="PSUM"))
        ps_mm = ctx.enter_context(tc.tile_pool(name="ps_mm", bufs=1, space="PSUM"))
        ps_z = ctx.enter_context(tc.tile_pool(name="ps_z", bufs=1, space="PSUM"))
        ps_rec = ctx.enter_context(tc.tile_pool(name="ps_rec", bufs=1, space="PSUM"))
        ps_wt = ctx.enter_context(tc.tile_pool(name="ps_wt", bufs=1, space="PSUM"))

        # ---------- conv-main inputs first: they gate the whole chain ----------
        xrep0 = big.tile([KK, B, T], F32R)
        nc.sync.dma_start(xrep0[:], xrep0_i[:])
        wmain_sb = const.tile([KK, H], F32R)
        nc.sync.dma_start(wmain_sb[:], w_main[:])
        cbm_sb = const.tile([H, 1], F32)
        nc.sync.dma_start(cbm_sb[:], cb_main[:])
        waux_sb = const.tile([DK, 128], F32R)
        nc.sync.dma_start(waux_sb[:], w_aux[:])
        wrec_sb = const.tile([DK, DH], F32R)
        nc.sync.dma_start(wrec_sb[:], w_rec[:])
        wdec_sb = const.tile([128, 8, DK], F32R)
        nc.sync.dma_start(wdec_sb[:], w_dec[:])
        cba_sb = const.tile([128, 1], F32)
        nc.sync.dma_start(cba_sb[:], cb_aux[:])
        cbr_sb = const.tile([128, 8], F32)
        nc.sync.dma_start(cbr_sb[:], cb_rec[:])
        decb_sb = const.tile([D, 1], F32)
        nc.sync.dma_start(decb_sb[:], dec_b[:])
        sel8_sb = const.tile([128, HSH], F32R)
        nc.sync.dma_start(sel8_sb[:], sel8[:])
        seld_sb = const.tile([D - 1, 128], F32R)
        nc.sync.dma_start(seld_sb[:], sel_d[:])
        self_sb = const.tile([128, D - 1], F32R)
        nc.sync.dma_start(self_sb[:], sel_f[:])
        selk_sb = const.tile([DK, KK, D], F32R)
        nc.sync.dma_start(selk_sb[:], sel_k[:])
        selr_sb = const.tile([HSH, 128], F32R)
        nc.sync.dma_start(selr_sb[:], sel_r[:])

        # ---------- im2col inputs (host-prepared, contiguous loads) ----------
        # on scalar so sync can start streaming W immediately
        xrep = big.tile([DK, B, T], F32R)
        nc.gpsimd.dma_start(xrep[:], xrep_i[:])
        xrrep = big.tile([DK, BSH, T], F32R)
        nc.gpsimd.dma_start(xrrep[:], xrrep_i[:])

        # ---------- conv main (d=0, all h), chunked over t ----------
        # each chunk completes whole mainT kc-columns, so the big matmul
        # starts as soon as the first chunks land
        pm = big.tile([H, B, P], F32)       # pooled main, (b,p) free order
        mainT_sb = big.tile([128, NKC, B], BF16)
        TC = 16                             # t per conv-main chunk (8 p, 4 kc)
        for c in range(16):                 # 15 full chunks + 1 tail
            tn = TC if c < 15 else T - 15 * TC   # last: 12 t -> 6 p, 3 kc
            pn, kn = tn // 2, tn // 4
            ps = ps_conv.tile([128, B * TC], F32, tag="ps")
            nc.tensor.matmul(ps[:H, 0:B * tn], wmain_sb[:],
                             xrep0[:, :, TC * c:TC * c + tn].transpose([0, 2, 1]),
                             start=True, stop=True)
            ym = ypool.tile([H, TC, B], F32, tag="ym")
            nc.scalar.activation(ym[:, 0:tn, :],
                                 ps[:H, 0:B * tn].rearrange("h (t b) -> h t b", b=B),
                                 ACTF.Relu, bias=cbm_sb[:])
            nc.vector.tensor_tensor(
                pm[:, :, 8 * c:8 * c + pn].transpose([0, 2, 1]),
                ym[:, 0:tn:2, :], ym[:, 1:tn:2, :], op=ALU.add)
            nc.vector.tensor_tensor(mainT_sb[0:64, 4 * c:4 * c + kn, :],
                                    ym[:, 0:tn:4, :], ym[:, 1:tn:4, :], op=ALU.add)
            nc.vector.tensor_tensor(mainT_sb[64:128, 4 * c:4 * c + kn, :],
                                    ym[:, 2:tn:4, :], ym[:, 3:tn:4, :], op=ALU.add)

        # ---------- conv aux (all d, this core's 8 h) ----------
        pa = big.tile([128, B, P], F32R)    # pooled aux [(d,h8), b, p]
        NB = 2                              # batches per aux conv chunk
        for c in range(B // NB):
            ps = ps_conv.tile([128, NB * T], F32, tag="ps")
            nc.tensor.matmul(ps[:], waux_sb[:],
                             xrep[:, NB * c:NB * (c + 1), :],
                             start=True, stop=True)
            ya = ypool.tile([128, NB, T], F32, tag="ya")
            nc.scalar.activation(ya[:], ps[:].rearrange("h (b t) -> h b t", b=NB),
                                 ACTF.Relu, bias=cba_sb[:])
            nc.vector.tensor_tensor(pa[:, NB * c:NB * (c + 1), :],
                                    ya[:, :, 0:T - 1:2], ya[:, :, 1:T:2], op=ALU.add)

        # ---------- big matmul: m = main @ Wsh ----------
        m_ps0 = ps_mm.tile([B, 504], F32, tag="m0")
        m_ps1 = ps_mm.tile([B, 504], F32, tag="m1")
        dma_engines = [nc.sync, nc.scalar]
        WMAC = 8                            # k-chunks per W macro-tile DMA
        kc = 0
        ti = 0
        while kc < NKC:
            nmac = min(WMAC, NKC - kc)
            wt = wpool.tile([128, WMAC, SSH], BF16, tag="w")
            dma_engines[ti % 2].dma_start(
                wt[:, 0:nmac, :], Wsh[:, kc:kc + nmac, :])
            for j in range(nmac):
                nc.tensor.matmul(m_ps0[:], mainT_sb[:, kc + j, :],
                                 wt[:, j, 0:504],
                                 start=(kc + j == 0), stop=(kc + j == NKC - 1))
                nc.tensor.matmul(m_ps1[:], mainT_sb[:, kc + j, :],
                                 wt[:, j, 504:SSH],
                                 start=(kc + j == 0), stop=(kc + j == NKC - 1))
            kc += nmac
            ti += 1
        nc.gpsimd.dma_start(main_out[:], pm[:])
        m_sb = big.tile([B, SSH], F32R)
        nc.scalar.copy(m_sb[:, 0:504], m_ps0[:])
        nc.scalar.copy(m_sb[:, 504:SSH], m_ps1[:])

        # ---------- scores partials + AllReduce ----------
        # m8[h8, b, p] = m[b, h8*P + p]; PE-broadcast to all 16 d-slots
        nc.scalar.dma_start(m_d[:], m_sb[:])
        m8 = big.tile([HSH, B, P], F32R)
        nc.sync.dma_start(m8[:], m_d[:].rearrange("b (h p) -> h b p", h=HSH))
        m_rep = big.tile([128, B, P], F32)
        sc_hb = const.tile([128, B], F32R)
        m8f = m8[:].rearrange("h b p -> h (b p)")
        mrf = m_rep[:].rearrange("q b p -> q (b p)")
        paf = pa[:].rearrange("q b p -> q (b p)")
        for c8 in range(8):
            cs = slice(504 * c8, 504 * (c8 + 1))
            mr_ps = ps_wt.tile([128, 504], F32, tag="wt")
            nc.tensor.matmul(mr_ps[:], selr_sb[:], m8f[:, cs],
                             start=True, stop=True)
            nc.scalar.copy(mrf[:, cs], mr_ps[:])
            nc.vector.tensor_tensor(mrf[:, cs], mrf[:, cs], paf[:, cs], op=ALU.mult)
            with nc.allow_low_precision(reason="f32r scores rounding"):
                nc.vector.tensor_reduce(
                    sc_hb[:, 4 * c8:4 * (c8 + 1)],
                    m_rep[:, 4 * c8:4 * (c8 + 1), :], axis=AX.X, op=ALU.add)
        # fold h8 on PE: scT[d', b] = sum_{(d,h8)} sel_f * sc_hb
        sc_ps = ps_wt.tile([D - 1, B], F32, tag="wt")
        nc.tensor.matmul(sc_ps[:], self_sb[:], sc_hb[:], start=True, stop=True)
        scT_sb = const.tile([D - 1, B], F32)
        nc.scalar.copy(scT_sb[:], sc_ps[:])
        nc.sync.dma_start(cc_in[:], scT_sb[:])
        nc.gpsimd.collective_compute(
            "AllReduce", ALU.add, replica_groups=[list(range(NC))],
            ins=[cc_in[:]], outs=[cc_out[:]])

        # ---------- recon path (fills the collective window) ----------
        z_sb = big.tile([DK, BSH, L + 4], F32R)   # zero-padded: z at t+4
        # f32r memset lacks an ISA encoding; zero the margins via ACT scale=0
        nc.scalar.activation(z_sb[:, :, 0:4], selk_sb[:, 0:4, 0:4],
                             ACTF.Copy, scale=0.0)
        nc.scalar.activation(z_sb[:, :, 256:260], selk_sb[:, 0:4, 0:4],
                             ACTF.Copy, scale=0.0)
        xrf = xrrep[:].rearrange("a b t -> a (b t)")
        for h2 in range(2):
            z_ps = ps_z.tile([DK, 504], F32, tag="z")
            for j in range(8):
                ps = ps_conv.tile([128, 504], F32, tag="ps")
                nc.tensor.matmul(ps[:], wrec_sb[:, 128 * j:128 * (j + 1)],
                                 xrf[:, 504 * h2:504 * (h2 + 1)],
                                 start=True, stop=True)
                yr = yrpool.tile([128, 504], F32R, tag="yr")
                nc.scalar.activation(yr[:], ps[:], ACTF.Relu,
                                     bias=cbr_sb[:, j:j + 1])
                nc.tensor.matmul(z_ps[:], wdec_sb[:, j, :], yr[:],
                                 start=(j == 0), stop=(j == 7))
            nc.scalar.copy(z_sb[:, 2 * h2:2 * h2 + 2, 4:4 + T],
                           z_ps[:].rearrange("a (b t) -> a b t", b=2))
        # k-fold via accumulating selector matmuls over shifted input windows
        rec_ps = ps_rec.tile([D, BSH, L], F32)
        for b4 in range(BSH):
            for k in range(KK):
                nc.tensor.matmul(rec_ps[:, b4, :], selk_sb[:, k, :],
                                 z_sb[:, b4, 4 - k:4 - k + L],
                                 start=(k == 0), stop=(k == KK - 1),
                                 skip_group_check=True)
        rec_sb = big.tile([D, BSH, L], F32)
        nc.vector.tensor_scalar_add(rec_sb[:], rec_ps[:], decb_sb[:])
        nc.sync.dma_start(recon_out[:], rec_sb[:])

        # ---------- softmax ----------
        scores = const.tile([B, D - 1], F32)
        nc.sync.dma_start(scores[:], cc_out[:].transpose([1, 0]))
        ex = const.tile([B, D - 1], F32)
        esum = const.tile([B, 1], F32)
        nc.scalar.activation(ex[:], scores[:], ACTF.Exp, accum_out=esum[:])
        rs = const.tile([B, 1], F32)
        nc.vector.reciprocal(rs[:], esum[:])
        attn_sb = const.tile([B, D - 1], F32)
        nc.vector.tensor_scalar_mul(attn_sb[:], ex[:], rs[:])
        nc.sync.dma_start(attn_out[:], attn_sb[:])
        nc.sync.dma_start(attnT_d[:].transpose([1, 0]), attn_sb[:].bitcast(F32R))
        attnT_sb = const.tile([D - 1, B], F32R)
        nc.sync.dma_start(attnT_sb[:], attnT_d[:])

        # ---------- weighted = sum_d attn[b,d] * aux ----------
        arep_ps = ps_conv.tile([128, B], F32, tag="ps")
        nc.tensor.matmul(arep_ps[:], seld_sb[:], attnT_sb[:], start=True, stop=True)
        arep_sb = const.tile([128, B], F32)
        nc.scalar.copy(arep_sb[:], arep_ps[:])
        # wsc = aux * attn (broadcast along p), chunked and pipelined
        NBW = 4                             # batches per selector-matmul chunk
        for c in range(B // NBW):
            nc.vector.tensor_tensor(
                pa[:, NBW * c:NBW * (c + 1), :], pa[:, NBW * c:NBW * (c + 1), :],
                arep_sb[:].unsqueeze(-1).broadcast_to([128, B, P])
                [:, NBW * c:NBW * (c + 1), :], op=ALU.mult)
            wt_ps = ps_wt.tile([HSH, NBW * P], F32, tag="wt")
            nc.tensor.matmul(wt_ps[:], sel8_sb[:],
                             pa[:, NBW * c:NBW * (c + 1), :],
                             start=True, stop=True)
            wchunk = wtpool.tile([HSH, NBW, P], F32, tag="wc")
            nc.scalar.copy(wchunk[:], wt_ps[:].rearrange("h (b p) -> h b p", b=NBW))
            nc.sync.dma_start(weighted_out[:, NBW * c:NBW * (c + 1), :], wchunk[:])

    nc.compile()
    return nc


def _prep_inputs(x, conv_w, conv_b, deconv_w, deconv_b, W_attn):
    """Build the per-core input maps (all numpy float32)."""
    x = np.ascontiguousarray(np.asarray(x, np.float32))
    conv_w = np.asarray(conv_w, np.float32)
    conv_b = np.asarray(conv_b, np.float32)
    deconv_w = np.asarray(deconv_w, np.float32)
    deconv_b = np.asarray(deconv_b, np.float32)
    W_attn = np.asarray(W_attn, np.float32)

    xT = np.ascontiguousarray(x.transpose(2, 0, 1))          # [D, B, L]
    # im2col stacks: xrep[(k,d), b, t] = x[b, t+k, d]
    xrep_i = np.ascontiguousarray(np.concatenate(
        [xT[:, :, k:k + T] for k in range(KK)], axis=0).reshape(DK, B, T))
    xrep0_i = np.ascontiguousarray(
        np.stack([xT[0, :, k:k + T] for k in range(KK)], axis=0))
    # rows permuted: s' = 64p + h  <->  s = 126h + p
    W_perm = np.ascontiguousarray(
        W_attn.reshape(H, P, S).transpose(1, 0, 2).reshape(S, S))

    # conv stationaries, (k,d) partition order; aux/main scaled by 0.5
    kd = np.zeros((DK, D, H), np.float32)                    # [(k,d), d2, h]
    for k in range(KK):
        for d in range(D):
            kd[16 * k + d, d, :] = conv_w[d, :, k]
    w_main = np.ascontiguousarray(0.5 * conv_w[0].T)          # [5, 64]
    w_rec = np.ascontiguousarray(kd.reshape(DK, DH))          # [80, 1024]

    wdec = np.zeros((DH, DK), np.float32)
    for d in range(D):
        for k in range(KK):
            wdec[d * H:(d + 1) * H, 16 * k + d] = deconv_w[d, :, k]
    w_dec = np.ascontiguousarray(wdec.reshape(8, 128, DK).transpose(1, 0, 2))

    cb_main = np.ascontiguousarray(0.5 * conv_b[0, :, None])  # [64,1]
    cb_rec = np.ascontiguousarray(conv_b.reshape(DH).reshape(8, 128).T)
    dec_b = np.ascontiguousarray(deconv_b[:, None])           # [16,1]

    # sel8[(d,h8), h8'] = [h8==h8'] for d>=1 else 0 -> [128, 8]
    sel8 = np.ascontiguousarray(np.concatenate(
        [np.zeros((HSH, HSH), np.float32),
         np.tile(np.eye(HSH, dtype=np.float32), (D - 1, 1))], axis=0))
    # sel_d[d', (d,h8)] = [d==d'+1] -> [15, 128]
    sel_d = np.zeros((D - 1, 128), np.float32)
    for dp in range(D - 1):
        sel_d[dp, (dp + 1) * HSH:(dp + 2) * HSH] = 1.0
    # sel_f[(d,h8), d'] = [d==d'+1] -> [128, 15]
    sel_f = np.ascontiguousarray(sel_d.T)
    # sel_r[h8', (d,h8)] = [h8==h8'] -> [8, 128] (PE broadcast of m8)
    sel_r = np.ascontiguousarray(np.tile(np.eye(HSH, dtype=np.float32), (1, D)))
    # sel_k[(k',d'), k, d] = [k'==k][d'==d] -> [80, 5, 16]
    sel_k = np.zeros((DK, KK, D), np.float32)
    for k in range(KK):
        for d in range(D):
            sel_k[16 * k + d, k, d] = 1.0

    in_maps = []
    for c in range(NC):
        wa = np.zeros((DK, 128), np.float32)
        for k in range(KK):
            for d in range(D):
                wa[16 * k + d, d * HSH:(d + 1) * HSH] = \
                    0.5 * conv_w[d, HSH * c:HSH * (c + 1), k]
        cb_aux = np.ascontiguousarray(
            0.5 * conv_b[:, HSH * c:HSH * (c + 1)].reshape(128)[:, None])
        xslice = xT[:, BSH * c:BSH * (c + 1), :]
        xrrep_i = np.ascontiguousarray(np.concatenate(
            [xslice[:, :, k:k + T] for k in range(KK)], axis=0).reshape(DK, BSH, T))
        in_maps.append({
            "xrep_i": xrep_i,
            "xrep0_i": xrep0_i,
            "xrrep_i": xrrep_i,
            "w_aux": wa,
            "w_main": w_main,
            "w_rec": w_rec,
            "w_dec": w_dec,
            "cb_aux": cb_aux,
            "cb_main": cb_main,
            "cb_rec": cb_rec,
            "dec_b": dec_b,
            "sel8": sel8,
            "sel_d": sel_d,
            "sel_f": sel_f,
            "sel_k": sel_k,
            "sel_r": sel_r,
            "Wsh": np.ascontiguousarray(
                W_perm[:, SSH * c:SSH * (c + 1)].astype(ml_dtypes.bfloat16)
                .reshape(NKC, 128, SSH).transpose(1, 0, 2)),
        })
    return in_maps


def _assemble(results):
    main = np.ascontiguousarray(
        results[0]["main_out"].transpose(1, 0, 2)).reshape(B, S)
    weighted = np.empty((B, H, P), np.float32)
    for c, r in enumerate(results):
        weighted[:, HSH * c:HSH * (c + 1), :] = r["weighted_out"].transpose(1, 0, 2)
    weighted = weighted.reshape(B, S)
    attn = results[0]["attn_out"][:, None, :]                  # [B,1,15]
    recon = np.concatenate(
        [r["recon_out"].transpose(1, 0, 2) for r in results], axis=0)
    out = np.concatenate([main[:, None, :], weighted[:, None, :]], axis=2)
    return out, attn, recon


def kernel(x, conv_w, conv_b, deconv_w, deconv_b, W_attn,
           _trace=False, _trace_kwargs=None):
    if "nc" not in _CACHED:
        _CACHED["nc"] = build_nc()
    nc = _CACHED["nc"]
    in_maps = _prep_inputs(x, conv_w, conv_b, deconv_w, deconv_b, W_attn)
    res = run_bass_kernel_spmd(nc, in_maps, list(range(NC)), trace=_trace,
                               **(_trace_kwargs or {}))
    outs = _assemble(res.results)
    if _trace:
        return outs, res
    return outs


if __name__ == "__main__":
    d = np.load("/root/problem/ref_cache.npz")
    out, attn, recon = kernel(d["x"], d["conv_w"], d["conv_b"],
                              d["deconv_w"], d["deconv_b"], d["W_attn"])
    for name, a, b in [("out", out, d["out"]), ("attn", attn, d["attn"]),
                       ("recon", recon, d["recon"])]:
        err = np.abs(a - b).max() / (np.abs(b).max() + 1e-30)
        print(f"{name}: rel_err {err:.3e}")
